# revision 1
# baseline (speedup 1.0000x reference)
"""Trainium2 Bass kernel for CachedMistralAttention prefill (B=1, S=2048, H=4096,
32 q heads / 8 kv heads GQA, rotate-half RoPE, causal SDPA).

Sharding: tensor-parallel over heads across 8 NeuronCores. Core c owns q heads
[4c, 4c+4) and kv head c (one GQA group), computes its partial output
projection attn @ wo[4c:4c+4], and the host sums the 8 partials.

Per-core dataflow (all matmuls bf16 with fp32 PSUM accumulation):
  phase A: qT/kT/vT = W.T @ hiddenT per s-block; RoPE applied on the fly
           (rotate-half via partition-offset DVE reads); v transposed to
           [s, d] via DMA-transpose for use as the PV stationary operand.
  phase B: per (head, 512-wide sq block): scoresT[sk, sq] = kT.T @ qT tiles,
           exp on ScalarE (scale=1/sqrt(d)) with causal masking (skip upper
           tiles, zero left strip + triangular mask on diagonal tiles),
           attn_unnormT[d, sq] = v.T @ expT and denom[1, sq] = ones.T @ expT
           accumulated on PSUM, reciprocal + partition_broadcast + DVE mult
           for the normalization.
  phase C: out[sq, m] = sum_h attnT_h.T @ wo_h accumulated in PSUM over
           heads, evacuated to SBUF and DMA'd out as fp32.

All DRAM inputs are pre-tiled on the host so each DMA reads per-partition
contiguous spans (few, large descriptors - DMA issue is the scarce resource).
"""

import math
from contextlib import ExitStack

import numpy as np
import ml_dtypes

import concourse.bacc as bacc
import concourse.mybir as mybir
import concourse.tile as tile
from concourse.bass_utils import run_bass_kernel_spmd

BF16 = mybir.dt.bfloat16
F32 = mybir.dt.float32
NPBF16 = ml_dtypes.bfloat16

S = 2048          # sequence length
H = 4096          # hidden size
D = 128           # head dim
NH = 4            # q heads per core (one GQA group)
NCORES = 8
KT = H // 128     # 32 contraction tiles for the projections
SB = 512          # phase A s-block width
NSB = S // SB     # 4
KG = 16           # kt-tiles per hidden-strip sub-DMA chunk
SQB = 512         # phase B sq-block width
NSQB = S // SQB   # 4
NSK = S // 128    # 16 sk tiles
INV_NORM = 1.0 / math.sqrt(D)
MAX_WAVELENGTH = 10000.0


def _build_program():
    nc = bacc.Bacc("TRN2", target_bir_lowering=False, debug=False,
                   num_devices=NCORES)

    # pre-tiled inputs: leading dim 128 = SBUF partition, free dims contiguous
    hid_d = nc.dram_tensor("hidP", [NSB, 128, KT * SB], BF16, kind="ExternalInput")
    wq_d = nc.dram_tensor("wqP", [128, KT * NH * D], BF16, kind="ExternalInput")
    wk_d = nc.dram_tensor("wkP", [128, KT * D], BF16, kind="ExternalInput")
    wv_d = nc.dram_tensor("wvP", [128, KT * D], BF16, kind="ExternalInput")
    wo_d = nc.dram_tensor("woP", [128, NH * H], BF16, kind="ExternalInput")
    cos_d = nc.dram_tensor("cosT", [D, S], F32, kind="ExternalInput")
    sinA_d = nc.dram_tensor("sinA", [D, S], F32, kind="ExternalInput")
    tri_d = nc.dram_tensor("trimask", [D, D], BF16, kind="ExternalInput")
    out_d = nc.dram_tensor("out", [S, H], F32, kind="ExternalOutput")

    with tile.TileContext(nc) as tc, ExitStack() as ctx:
        # pools
        wqo_p = ctx.enter_context(tc.tile_pool(name="wqo", bufs=1))
        const_p = ctx.enter_context(tc.tile_pool(name="const", bufs=1))
        qkv_p = ctx.enter_context(tc.tile_pool(name="qkv", bufs=1))
        ps_p = None  # created per phase

        wq_sb = wqo_p.tile([128, KT, NH * D], BF16, tag="wqo")
        cos_sb = const_p.tile([D, S], F32, tag="cos")
        sinA_sb = const_p.tile([D, S], F32, tag="sin")
        tri_sb = const_p.tile([D, D], BF16, tag="tri")
        ones_sb = const_p.tile([128, 1], BF16, tag="ones")
        nc.vector.memset(ones_sb, 1.0)

        # persistent activations
        qT_sb = qkv_p.tile([128, NH, S], BF16, tag="qT")    # [d, h, s]
        kT_sb = qkv_p.tile([128, S], BF16, tag="kT")        # [d, s]
        vT_sb = qkv_p.tile([128, S], BF16, tag="vT")        # [d, s]
        v_sb = qkv_p.tile([128, NSK, D], BF16, tag="v")     # [s%128, skt, d]
        attnT_sb = qkv_p.tile([128, NH, S], BF16, tag="attnT")

        wq_v = wq_sb.rearrange("p kt n -> p (kt n)")

        # ---------------- phase A: projections + RoPE -----------------
        pha = ExitStack()
        psa_p = pha.enter_context(tc.tile_pool(name="psa", bufs=8, space="PSUM"))
        hid_p = pha.enter_context(tc.tile_pool(name="hid", bufs=2))
        wkv_p = pha.enter_context(tc.tile_pool(name="wkv", bufs=1))
        rope_p = pha.enter_context(tc.tile_pool(name="rope", bufs=3))
        wk_sb = wkv_p.tile([128, KT, D], BF16, tag="wk")
        wv_sb = wkv_p.tile([128, KT, D], BF16, tag="wv")
        for sb in range(NSB):
            ssl = slice(sb * SB, (sb + 1) * SB)
            hid_sb = hid_p.tile([128, KT, SB], BF16, tag="hid")
            hid_v = hid_sb.rearrange("p kt s -> p (kt s)")
            # strip DMA in kt-group chunks (contiguous on both sides);
            # finer chunks on the first strip so the first matmul starts early
            bounds = [0, 2, 4, 8, 16, 24, KT] if sb == 0 else \
                     list(range(0, KT + 1, KG))
            for g0, g1 in zip(bounds[:-1], bounds[1:]):
                nc.sync.dma_start(
                    out=hid_v[:, g0 * SB:g1 * SB],
                    in_=hid_d[:][sb, :, g0 * SB:g1 * SB])
                if sb == 0:
                    gb = NH * D
                    nc.scalar.dma_start(
                        out=wq_v[:, g0 * gb:g1 * gb],
                        in_=wq_d[:][:, g0 * gb:g1 * gb])
            if sb == 0:
                nc.scalar.dma_start(out=wk_sb.rearrange("p kt n -> p (kt n)"),
                                    in_=wk_d[:])
                nc.scalar.dma_start(out=wv_sb.rearrange("p kt n -> p (kt n)"),
                                    in_=wv_d[:])
                nc.scalar.dma_start(out=cos_sb, in_=cos_d[:])
                nc.scalar.dma_start(out=sinA_sb, in_=sinA_d[:])
                nc.scalar.dma_start(out=tri_sb, in_=tri_d[:])
            for t in range(NH + 2):  # 0..3 q heads, 4 = k, 5 = v
                ps = psa_p.tile([128, SB], F32, tag="psa")
                for kt in range(KT):
                    if t < NH:
                        lhsT = wq_sb[:, kt, t * D:(t + 1) * D]
                    elif t == NH:
                        lhsT = wk_sb[:, kt, :]
                    else:
                        lhsT = wv_sb[:, kt, :]
                    nc.tensor.matmul(ps, lhsT, hid_sb[:, kt, :],
                                     start=(kt == 0), stop=(kt == KT - 1))
                if t <= NH:
                    # RoPE: x*cos + rot(x)*sin, rot = [-x2, x1] (partition halves)
                    t1 = rope_p.tile([128, SB], F32, tag="t1")
                    t2 = rope_p.tile([128, SB], F32, tag="t2")
                    nc.vector.tensor_mul(t1, ps, cos_sb[:, ssl])
                    nc.vector.tensor_mul(t2[0:64, :], ps[64:128, :],
                                         sinA_sb[0:64, ssl])
                    nc.vector.tensor_mul(t2[64:128, :], ps[0:64, :],
                                         sinA_sb[64:128, ssl])
                    dst = qT_sb[:, t, ssl] if t < NH else kT_sb[:, ssl]
                    nc.vector.tensor_add(dst, t1, t2)
                else:
                    # v: evacuate to bf16; transposed in one batch at end of
                    # phase A (minimizes DMA xbar-mode transitions)
                    nc.scalar.copy(vT_sb[:, ssl], ps)

        for skt in range(NSK):
            nc.sync.dma_start_transpose(
                out=v_sb[:, skt, :], in_=vT_sb[:, skt * 128:(skt + 1) * 128])
        pha.close()

        # ---------------- phase B: attention per head -----------------
        phb = ExitStack()
        ps_p = ctx.enter_context(tc.tile_pool(name="ps", bufs=5, space="PSUM"))
        psat_p = phb.enter_context(tc.tile_pool(name="psat", bufs=2, space="PSUM"))
        psden_p = phb.enter_context(tc.tile_pool(name="psden", bufs=1, space="PSUM"))
        exp_p = phb.enter_context(tc.tile_pool(name="expp", bufs=12))
        rec_p = phb.enter_context(tc.tile_pool(name="recp", bufs=4))
        for b in range(NSQB):
            for h in range(NH):
                qsl = slice(b * SQB, (b + 1) * SQB)
                nsk = (b + 1) * (SQB // 128)
                ps_at = psat_p.tile([128, SQB], F32, tag="at")
                ps_den = psden_p.tile([1, SQB], F32, tag="den")
                for skt in range(nsk):
                    # diagonal blocks (j >= 0): columns sq < skt*128 are fully
                    # causal-masked - skip them in QK/PV/den and zero them in e
                    j = skt - b * (SQB // 128)
                    lo = max(j, 0) * 128      # first live column in this block
                    ps_sc = ps_p.tile([128, SQB], F32, tag="ps")
                    nc.tensor.matmul(ps_sc[:, lo:],
                                     kT_sb[:, skt * 128:(skt + 1) * 128],
                                     qT_sb[:, h, b * SQB + lo:(b + 1) * SQB],
                                     start=True, stop=True)
                    e = exp_p.tile([128, SQB], BF16, tag="e")
                    nc.scalar.activation(e[:, lo:], ps_sc[:, lo:],
                                         mybir.ActivationFunctionType.Exp,
                                         scale=INV_NORM)
                    if j >= 0:
                        # triangular mask on the [128,128] diagonal tile
                        nc.vector.tensor_mul(e[:, lo:lo + 128],
                                             e[:, lo:lo + 128], tri_sb)
                    nc.tensor.matmul(ps_at[:, lo:], v_sb[:, skt, :], e[:, lo:],
                                     start=(skt == 0), stop=(skt == nsk - 1))
                    nc.tensor.matmul(ps_den[:, lo:], ones_sb, e[:, lo:],
                                     start=(skt == 0), stop=(skt == nsk - 1))
                rec = rec_p.tile([1, SQB], F32, tag="rec")
                nc.vector.reciprocal(rec, ps_den)
                recb = rec_p.tile([128, SQB], F32, tag="recb")
                nc.gpsimd.partition_broadcast(recb, rec)
                nc.vector.tensor_mul(attnT_sb[:, h, qsl], ps_at, recb)

        phb.close()

        # ---------------- phase C: output projection ------------------
        out_p = ctx.enter_context(tc.tile_pool(name="outp", bufs=2))
        wo_sb = wqo_p.tile([128, NH, H], BF16, tag="wqo")
        nc.scalar.dma_start(out=wo_sb.rearrange("p h m -> p (h m)"), in_=wo_d[:])
        NMB = H // SQB  # 8 column blocks of 512
        HB = NMB // 2   # 4 blocks per half-row
        for sqt in range(S // 128):
            for half in range(2):
                o_sb = out_p.tile([128, HB * SQB], F32, tag="o")
                pss = [ps_p.tile([128, SQB], F32, tag="ps",
                                 name=f"pso_{sqt}_{half}_{i}")
                       for i in range(HB)]
                for hh in range(NH):
                    lhsT = attnT_sb[:, hh, sqt * 128:(sqt + 1) * 128]
                    for i in range(HB):
                        mb = half * HB + i
                        nc.tensor.matmul(pss[i], lhsT,
                                         wo_sb[:, hh, mb * SQB:(mb + 1) * SQB],
                                         start=(hh == 0), stop=(hh == NH - 1))
                last = (sqt == S // 128 - 1)
                for i in range(HB):
                    if i % 2 == 0:
                        nc.scalar.copy(o_sb[:, i * SQB:(i + 1) * SQB], pss[i])
                    else:
                        nc.vector.tensor_copy(o_sb[:, i * SQB:(i + 1) * SQB],
                                              pss[i])
                    if last:
                        # fine-grained tail DMAs so the drain isn't gated on
                        # one big final transfer
                        mb = half * HB + i
                        nc.sync.dma_start(
                            out=out_d[:][sqt * 128:(sqt + 1) * 128,
                                         mb * SQB:(mb + 1) * SQB],
                            in_=o_sb[:, i * SQB:(i + 1) * SQB])
                if not last:
                    nc.sync.dma_start(
                        out=out_d[:][sqt * 128:(sqt + 1) * 128,
                                     half * HB * SQB:(half + 1) * HB * SQB],
                        in_=o_sb)

    nc.compile()
    return nc


_NC = None


def _get_nc():
    global _NC
    if _NC is None:
        _NC = _build_program()
    return _NC


def _host_tables():
    pos = np.arange(S, dtype=np.float32)
    inv_freq = 1.0 / (MAX_WAVELENGTH ** (np.arange(0, D, 2, dtype=np.float32) / D))
    freq = np.einsum('i,j->ij', pos, inv_freq)          # [S, 64]
    emb = np.concatenate([freq, freq], axis=1)          # [S, 128]
    cosT = np.ascontiguousarray(np.cos(emb).T).astype(np.float32)   # [128, S]
    sinT = np.sin(emb).T.astype(np.float32)
    sinA = sinT.copy()
    sinA[:64] = -sinT[:64]
    sinA = np.ascontiguousarray(sinA)
    tri = np.triu(np.ones((D, D), dtype=np.float32)).astype(NPBF16)  # p<=f keep
    return cosT, sinA, tri


def _prepare_in_maps(hidden_states, wq, wk, wv, wo):
    hs = np.asarray(hidden_states, dtype=np.float32)[0]        # [S, H]
    wq = np.asarray(wq, dtype=np.float32)                      # [H, 32, 128]
    wk = np.asarray(wk, dtype=np.float32)                      # [H, 8, 128]
    wv = np.asarray(wv, dtype=np.float32)
    wo = np.asarray(wo, dtype=np.float32)                      # [32, 128, H]

    # hidP[sb, p, kt*SB + s] = hiddenT[kt*128 + p, sb*SB + s]
    hidT = hs.T.astype(NPBF16)                                 # [H, S]
    hidP = np.ascontiguousarray(
        hidT.reshape(KT, 128, NSB, SB).transpose(2, 1, 0, 3).reshape(
            NSB, 128, KT * SB))
    cosT, sinA, tri = _host_tables()

    def ptile(w2d):  # [H, N] -> [128, KT*N] with (p, kt*N+n) = w2d[kt*128+p, n]
        n = w2d.shape[1]
        return np.ascontiguousarray(
            w2d.reshape(KT, 128, n).transpose(1, 0, 2).reshape(128, KT * n))

    in_maps = []
    for c in range(NCORES):
        wq_c = wq[:, NH * c:NH * (c + 1), :].reshape(H, NH * D).astype(NPBF16)
        wk_c = wk[:, c, :].astype(NPBF16)
        wv_c = wv[:, c, :].astype(NPBF16)
        wo_c = wo[NH * c:NH * (c + 1)].reshape(NH * D, H).astype(NPBF16)
        woP = np.ascontiguousarray(
            wo_c.reshape(NH, 128, H).transpose(1, 0, 2).reshape(128, NH * H))
        in_maps.append({
            "hidP": hidP,
            "wqP": ptile(wq_c),
            "wkP": ptile(wk_c),
            "wvP": ptile(wv_c),
            "woP": woP,
            "cosT": cosT,
            "sinA": sinA,
            "trimask": tri,
        })
    return in_maps


def _run(in_maps, **kwargs):
    return run_bass_kernel_spmd(_get_nc(), in_maps,
                                core_ids=list(range(NCORES)), **kwargs)


def _gather(res):
    out = np.zeros((S, H), dtype=np.float32)
    for c in range(NCORES):
        out += np.asarray(res.results[c]["out"], dtype=np.float32)
    return out[None]


def kernel(hidden_states, attention_mask=None, wq=None, wk=None, wv=None, wo=None):
    in_maps = _prepare_in_maps(hidden_states, wq, wk, wv, wo)
    return _gather(_run(in_maps))



# revision 19
# speedup vs baseline: 1.0489x; 1.0489x over previous
"""Trainium2 Bass kernel for CachedMistralAttention prefill (B=1, S=2048, H=4096,
32 q heads / 8 kv heads GQA, rotate-half RoPE, causal SDPA).

Sharding: tensor-parallel over heads across 8 NeuronCores. Core c owns q heads
[4c, 4c+4) and kv head c (one GQA group), computes its partial output
projection attn @ wo[4c:4c+4] in bf16, and the host sums the 8 partials.

v2 over the phase-sequential baseline:
  - softmax denominators via M=1-output matmuls (exp tile stationary, ones
    moving) instead of 512-wide ones-stationary matmuls: removes ~27us of
    tensor-engine streaming; normalization via reciprocal + PE transpose +
    per-chunk partition_broadcast.
  - attention block b's QK/exp/PV work is interleaved into projection window
    b+1 (and block 3 into the output-projection window) so the exp-bound
    scalar engine never stalls the tensor engine; PV units lag their QK units
    by one projection chain so exps are always ready.
  - PE warmup matmuls during the initial DMA wait (p-state ramp).
  - RoPE rotate-half multiplies on gpsimd, v/output evacuations off the
    scalar engine, output stored bf16.
"""

import math
import os
from contextlib import ExitStack

import numpy as np
import ml_dtypes

import concourse.bacc as bacc
import concourse.mybir as mybir
import concourse.tile as tile
from concourse.bass_utils import run_bass_kernel_spmd

BF16 = mybir.dt.bfloat16
F32 = mybir.dt.float32
NPBF16 = ml_dtypes.bfloat16

S = 2048          # sequence length
H = 4096          # hidden size
D = 128           # head dim
NH = 4            # q heads per core (one GQA group)
NCORES = 8
KT = H // 128     # 32 contraction tiles for the projections
SB = 512          # phase A s-block width
NSB = S // SB     # 4
KG = 16           # kt-tiles per hidden-strip sub-DMA chunk
SQB = 512         # attention sq-block width
NSQB = S // SQB   # 4
NSK = S // 128    # 16 sk tiles
INV_NORM = 1.0 / math.sqrt(D)
MAX_WAVELENGTH = 10000.0


def _build_program():
    nc = bacc.Bacc("TRN2", target_bir_lowering=False, debug=False,
                   num_devices=NCORES)

    hid_d = nc.dram_tensor("hidP", [NSB, 128, KT * SB], BF16, kind="ExternalInput")
    wq_d = nc.dram_tensor("wqP", [128, KT * NH * D], BF16, kind="ExternalInput")
    wk_d = nc.dram_tensor("wkP", [128, KT * D], BF16, kind="ExternalInput")
    wv_d = nc.dram_tensor("wvP", [128, KT * D], BF16, kind="ExternalInput")
    wo_d = nc.dram_tensor("woP", [128, NH * H], BF16, kind="ExternalInput")
    cos_d = nc.dram_tensor("cosT", [D, S], F32, kind="ExternalInput")
    sinA_d = nc.dram_tensor("sinA", [D, S], F32, kind="ExternalInput")
    tri_d = nc.dram_tensor("trimask", [D, D], BF16, kind="ExternalInput")
    ident_d = nc.dram_tensor("ident", [128, 128], F32, kind="ExternalInput")
    out_d = nc.dram_tensor("out", [S, H], BF16, kind="ExternalOutput")

    with tile.TileContext(nc) as tc, ExitStack() as ctx:
        wqo_p = ctx.enter_context(tc.tile_pool(name="wqo", bufs=1))
        const_p = ctx.enter_context(tc.tile_pool(name="const", bufs=1))
        qkv_p = ctx.enter_context(tc.tile_pool(name="qkv", bufs=1))
        hid_p = ctx.enter_context(tc.tile_pool(name="hid", bufs=2))
        wkv_p = ctx.enter_context(tc.tile_pool(name="wkv", bufs=1))
        rope_p = ctx.enter_context(tc.tile_pool(name="rope", bufs=2))
        e_p = ctx.enter_context(tc.tile_pool(name="expp", bufs=8))
        rec_p = ctx.enter_context(tc.tile_pool(name="recp", bufs=2))
        o_p = ctx.enter_context(tc.tile_pool(name="outp", bufs=2))  # 8KB
        # PSUM: psS 3 banks + psAT 1 + psDEN 1 + psRT 1 = 6, phase A adds 2,
        # phase C adds 2 (A closed first) -> always <= 8 banks.
        psS_p = ctx.enter_context(tc.tile_pool(name="psS", bufs=2, space="PSUM"))
        psAT_p = ctx.enter_context(tc.tile_pool(name="psAT", bufs=1, space="PSUM"))
        psDEN_p = ctx.enter_context(tc.tile_pool(name="psDEN", bufs=1, space="PSUM"))
        psRT_p = ctx.enter_context(tc.tile_pool(name="psRT", bufs=1, space="PSUM"))

        wq_sb = wqo_p.tile([128, KT, NH * D], BF16, tag="wqo")
        cos_sb = const_p.tile([D, S], F32, tag="cos")
        sinA_sb = const_p.tile([D, S], F32, tag="sin")
        tri_sb = const_p.tile([D, D], BF16, tag="tri")
        ident_sb = const_p.tile([128, 128], F32, tag="ident")
        ones_sb = const_p.tile([128, 1], BF16, tag="ones")
        warm_sb = const_p.tile([128, SB], BF16, tag="warm")
        nc.vector.memset(ones_sb, 1.0)
        nc.vector.memset(warm_sb, 0.0)

        qT_sb = qkv_p.tile([128, NH, S], BF16, tag="qT")    # [d, h, s]
        kT_sb = qkv_p.tile([128, S], BF16, tag="kT")        # [d, s]
        vT_sb = qkv_p.tile([128, S], BF16, tag="vT")        # [d, s]
        v_sb = qkv_p.tile([128, NSK, D], BF16, tag="v")     # [s%128, skt, d]
        attnT_sb = qkv_p.tile([128, NH, S], BF16, tag="attnT")

        wq_v = wq_sb.rearrange("p kt n -> p (kt n)")

        pha = ExitStack()
        psA_p = pha.enter_context(tc.tile_pool(name="psA", bufs=3, space="PSUM"))
        wk_sb = wkv_p.tile([128, KT, D], BF16, tag="wk")
        wv_sb = wkv_p.tile([128, KT, D], BF16, tag="wv")

        hid_tiles = {}

        # -------- PE warmup: keep PE busy through the initial DMA wait ------
        wps = psS_p.tile([1, SB], F32, tag="ps", name="warmps")
        for _ in range(6):
            nc.tensor.matmul(wps, ones_sb, warm_sb, start=True, stop=True)

        # ----------------------- unit builders -----------------------------
        def a_dma(sb):
            hid_sb = hid_p.tile([128, KT, SB], BF16, tag="hid",
                                name=f"hid_{sb}")
            hid_tiles[sb] = hid_sb
            hid_v = hid_sb.rearrange("p kt s -> p (kt s)")
            bounds = [0, 2, 4, 8, 16, 24, KT] if sb == 0 else \
                     list(range(0, KT + 1, KG))
            for g0, g1 in zip(bounds[:-1], bounds[1:]):
                nc.sync.dma_start(
                    out=hid_v[:, g0 * SB:g1 * SB],
                    in_=hid_d[:][sb, :, g0 * SB:g1 * SB])
                if sb == 0:
                    gb = NH * D
                    nc.scalar.dma_start(
                        out=wq_v[:, g0 * gb:g1 * gb],
                        in_=wq_d[:][:, g0 * gb:g1 * gb])
            if sb == 0:
                nc.scalar.dma_start(out=wk_sb.rearrange("p kt n -> p (kt n)"),
                                    in_=wk_d[:])
                nc.scalar.dma_start(out=wv_sb.rearrange("p kt n -> p (kt n)"),
                                    in_=wv_d[:])
                nc.scalar.dma_start(out=cos_sb, in_=cos_d[:])
                nc.scalar.dma_start(out=sinA_sb, in_=sinA_d[:])
                nc.scalar.dma_start(out=tri_sb, in_=tri_d[:])
                nc.scalar.dma_start(out=ident_sb, in_=ident_d[:])

        def a_chain(sb, t):
            ssl = slice(sb * SB, (sb + 1) * SB)
            hid_sb = hid_tiles[sb]
            ps = psA_p.tile([128, SB], F32, tag="psa", name=f"psa_{sb}_{t}")
            for kt in range(KT):
                if t < NH:
                    lhsT = wq_sb[:, kt, t * D:(t + 1) * D]
                elif t == NH:
                    lhsT = wk_sb[:, kt, :]
                else:
                    lhsT = wv_sb[:, kt, :]
                nc.tensor.matmul(ps, lhsT, hid_sb[:, kt, :],
                                 start=(kt == 0), stop=(kt == KT - 1))
            if t <= NH:
                # Act evacuates a partition-rotated copy so the PSUM bank
                # recycles fast and t2's mul has aligned SBUF partitions
                pcr = rope_p.tile([128, SB], F32, tag="pc", name=f"pc_{sb}_{t}")
                pc = rope_p.tile([128, SB], F32, tag="pn", name=f"pn_{sb}_{t}")
                nc.scalar.copy(pcr[0:64, :], ps[64:128, :])
                nc.scalar.copy(pcr[64:128, :], ps[0:64, :])
                nc.scalar.copy(pc, ps)
                # RoPE: x*cos + rot(x)*sin, rot = [-x2, x1] (partition halves)
                t1 = rope_p.tile([128, SB], F32, tag="t1", name=f"t1_{sb}_{t}")
                t2 = rope_p.tile([128, SB], F32, tag="t2", name=f"t2_{sb}_{t}")
                nc.vector.tensor_mul(t1, pc, cos_sb[:, ssl])
                nc.vector.tensor_mul(t2, pcr, sinA_sb[:, ssl])
                dst = qT_sb[:, t, ssl] if t < NH else kT_sb[:, ssl]
                nc.vector.tensor_add(dst, t1, t2)
            else:
                nc.scalar.copy(vT_sb[:, ssl], ps)

        def v_trans(sb):
            for skt in range(sb * 4, sb * 4 + 4):
                nc.sync.dma_start_transpose(
                    out=v_sb[:, skt, :],
                    in_=vT_sb[:, skt * 128:(skt + 1) * 128])

        # ---- attention units for block b: QK stream and (lagging) PV stream
        estore = {}
        atden = {}

        def make_b_streams(b):
            nsk = (b + 1) * (SQB // 128)
            qks, pvs = [], []
            for h in range(NH):
                for skt in range(nsk):
                    j = skt - b * 4
                    lo = max(j, 0) * 128

                    def qk(h=h, skt=skt, j=j, lo=lo, b=b):
                        ps_sc = psS_p.tile([128, SQB], F32, tag="ps",
                                           name=f"ps_{b}_{h}_{skt}")
                        nc.tensor.matmul(
                            ps_sc[:, lo:],
                            kT_sb[:, skt * 128:(skt + 1) * 128],
                            qT_sb[:, h, b * SQB + lo:(b + 1) * SQB],
                            start=True, stop=True)
                        e = e_p.tile([128, SQB], BF16, tag="e",
                                     name=f"e_{b}_{h}_{skt}")
                        nc.scalar.activation(e[:, lo:], ps_sc[:, lo:],
                                             mybir.ActivationFunctionType.Exp,
                                             scale=INV_NORM)
                        if j >= 0:
                            nc.vector.tensor_mul(e[:, lo:lo + 128],
                                                 e[:, lo:lo + 128], tri_sb)
                        estore[(b, h, skt)] = e
                    qks.append(qk)

                for skt in range(nsk):
                    lo = max(skt - b * 4, 0) * 128

                    def pv(h=h, skt=skt, lo=lo, b=b, nsk=nsk):
                        if skt == 0:
                            atden[(b, h)] = (
                                psAT_p.tile([128, SQB], F32, tag="at",
                                            name=f"at_{b}_{h}"),
                                psDEN_p.tile([128, 4, NSK], F32, tag="den",
                                             name=f"den_{b}_{h}"))
                        ps_at, ps_den = atden[(b, h)]
                        e = estore.pop((b, h, skt))
                        nc.tensor.matmul(ps_at[:, lo:], v_sb[:, skt, :],
                                         e[:, lo:], start=(skt == 0),
                                         stop=(skt == nsk - 1))
                        # denominator columns: exp tile stationary, ones
                        # moving -> [128,1] outputs, ~free on the PE
                        for c in range(lo // 128, 4):
                            nc.tensor.matmul(
                                ps_den[:, c, skt:skt + 1],
                                e[:, c * 128:(c + 1) * 128], ones_sb,
                                start=True, stop=True)
                    pvs.append(pv)

                def norm(h=h, b=b):
                    ps_at, ps_den = atden.pop((b, h))
                    denS = rec_p.tile([128, 4], F32, tag="rec",
                                      name=f"denS_{b}_{h}")
                    for c in range(4):
                        nc.vector.tensor_reduce(
                            denS[:, c:c + 1], ps_den[:, c, 0:b * 4 + c + 1],
                            mybir.AxisListType.X, mybir.AluOpType.add)
                    for c in range(4):
                        recT = psRT_p.tile([1, 128], F32, tag="rt",
                                           name=f"rt_{b}_{h}_{c}")
                        nc.tensor.transpose(recT, denS[:, c:c + 1], ident_sb)
                        recR = rec_p.tile([1, 128], F32, tag="recr",
                                          name=f"recr_{b}_{h}_{c}")
                        nc.vector.reciprocal(recR, recT)
                        recb = rec_p.tile([128, 128], F32, tag="recb",
                                          name=f"recb_{b}_{h}_{c}")
                        nc.gpsimd.partition_broadcast(recb, recR)
                        nc.vector.tensor_mul(
                            attnT_sb[:, h,
                                     b * SQB + c * 128:b * SQB + (c + 1) * 128],
                            ps_at[:, c * 128:(c + 1) * 128], recb)
                pvs.append(norm)
            return qks, pvs

        def window(chains, qks, pvs):
            """Interleave attention units into a window of big PE chains.
            QK batch g is emitted after chain g; PV batch g is emitted one
            chain later (alternating with QK batch g+1), so the exps a PV
            needs always have a full chain's time to complete."""
            K = len(chains)
            for g in range(K):
                chains[g]()
                q0, q1 = g * len(qks) // K, (g + 1) * len(qks) // K
                p0, p1 = ((g - 1) * len(pvs) // K if g else 0,
                          g * len(pvs) // K)
                qb, pb = qks[q0:q1], pvs[p0:p1]
                for i in range(max(len(qb), len(pb))):
                    if i < len(pb):
                        pb[i]()
                    if i < len(qb):
                        qb[i]()
            for u in pvs[(K - 1) * len(pvs) // K:]:
                u()

        # ------------------------- emission --------------------------------
        a_dma(0)
        a_dma(1)
        for t in range(NH + 2):
            a_chain(0, t)
        v_trans(0)

        for sb in range(1, NSB):
            if sb < NSB - 1:
                a_dma(sb + 1)
            if os.environ.get("SKIP_B", "0") == "1":
                qks, pvs = [], []
            else:
                qks, pvs = make_b_streams(sb - 1)
            chains = [lambda sb=sb, t=t: a_chain(sb, t) for t in range(NH + 2)]
            window(chains, qks, pvs)
            v_trans(sb)
            if sb == NSB - 1:
                # wo reuses wq's SBUF slot; DMA waits on the last wq read
                wo_sb = wqo_p.tile([128, NH, H], BF16, tag="wqo")
                nc.scalar.dma_start(
                    out=wo_sb.rearrange("p h m -> p (h m)"), in_=wo_d[:])

        pha.close()

        # ----------------- phase C with B3 interleaved ---------------------
        phc = ExitStack()
        psC_p = phc.enter_context(tc.tile_pool(name="psC", bufs=2, space="PSUM"))

        def c_unit(sqt, half, engines):
            o_sb = o_p.tile([128, 4 * SQB], BF16, tag="o",
                            name=f"o_{sqt}_{half}")
            last = sqt == S // 128 - 1
            for i in range(4):
                mb = half * 4 + i
                ps = psC_p.tile([128, SQB], F32, tag="psc",
                                name=f"psc_{sqt}_{half}_{i}")
                for hh in range(NH):
                    nc.tensor.matmul(ps,
                                     attnT_sb[:, hh, sqt * 128:(sqt + 1) * 128],
                                     wo_sb[:, hh, mb * SQB:(mb + 1) * SQB],
                                     start=(hh == 0), stop=(hh == NH - 1))
                eng = engines[i % len(engines)]
                if eng is nc.scalar:
                    eng.copy(o_sb[:, i * SQB:(i + 1) * SQB], ps)
                else:
                    eng.tensor_copy(o_sb[:, i * SQB:(i + 1) * SQB], ps)
                if last:
                    nc.sync.dma_start(
                        out=out_d[:][sqt * 128:(sqt + 1) * 128,
                                     mb * SQB:(mb + 1) * SQB],
                        in_=o_sb[:, i * SQB:(i + 1) * SQB])
            if not last:
                nc.sync.dma_start(
                    out=out_d[:][sqt * 128:(sqt + 1) * 128,
                                 half * 4 * SQB:(half + 1) * 4 * SQB],
                    in_=o_sb)

        if os.environ.get("SKIP_B", "0") == "1":
            qks, pvs = [], []
        else:
            qks, pvs = make_b_streams(NSB - 1)
        if os.environ.get("SKIP_C", "0") == "1":
            qks_ = qks
            for u in qks_:
                u()
            for u in pvs:
                u()
            phc.close()
            nc.compile()
            return nc
        eng_early = [nc.vector, nc.vector, nc.vector, nc.scalar]
        c_chains = [lambda sqt=sqt, half=half: c_unit(sqt, half, eng_early)
                    for sqt in range(12) for half in range(2)]
        window(c_chains, qks, pvs)
        eng_late = [nc.scalar, nc.vector, nc.vector, nc.scalar]
        for sqt in range(12, 16):
            for half in range(2):
                c_unit(sqt, half, eng_late)
        phc.close()

    nc.compile()
    return nc


_NC = None


def _get_nc():
    global _NC
    if _NC is None:
        _NC = _build_program()
    return _NC


def _host_tables():
    pos = np.arange(S, dtype=np.float32)
    inv_freq = 1.0 / (MAX_WAVELENGTH ** (np.arange(0, D, 2, dtype=np.float32) / D))
    freq = np.einsum('i,j->ij', pos, inv_freq)          # [S, 64]
    emb = np.concatenate([freq, freq], axis=1)          # [S, 128]
    cosT = np.ascontiguousarray(np.cos(emb).T).astype(np.float32)   # [128, S]
    sinT = np.sin(emb).T.astype(np.float32)
    sinA = sinT.copy()
    sinA[:64] = -sinT[:64]
    sinA = np.ascontiguousarray(sinA)
    tri = np.triu(np.ones((D, D), dtype=np.float32)).astype(NPBF16)  # p<=f keep
    return cosT, sinA, tri


def _prepare_in_maps(hidden_states, wq, wk, wv, wo):
    hs = np.asarray(hidden_states, dtype=np.float32)[0]        # [S, H]
    wq = np.asarray(wq, dtype=np.float32)                      # [H, 32, 128]
    wk = np.asarray(wk, dtype=np.float32)                      # [H, 8, 128]
    wv = np.asarray(wv, dtype=np.float32)
    wo = np.asarray(wo, dtype=np.float32)                      # [32, 128, H]

    hidT = hs.T.astype(NPBF16)                                 # [H, S]
    hidP = np.ascontiguousarray(
        hidT.reshape(KT, 128, NSB, SB).transpose(2, 1, 0, 3).reshape(
            NSB, 128, KT * SB))
    cosT, sinA, tri = _host_tables()
    ident = np.eye(128, dtype=np.float32)

    def ptile(w2d):  # [H, N] -> [128, KT*N] with (p, kt*N+n) = w2d[kt*128+p, n]
        n = w2d.shape[1]
        return np.ascontiguousarray(
            w2d.reshape(KT, 128, n).transpose(1, 0, 2).reshape(128, KT * n))

    in_maps = []
    for c in range(NCORES):
        wq_c = wq[:, NH * c:NH * (c + 1), :].reshape(H, NH * D).astype(NPBF16)
        wk_c = wk[:, c, :].astype(NPBF16)
        wv_c = wv[:, c, :].astype(NPBF16)
        wo_c = wo[NH * c:NH * (c + 1)].reshape(NH * D, H).astype(NPBF16)
        woP = np.ascontiguousarray(
            wo_c.reshape(NH, 128, H).transpose(1, 0, 2).reshape(128, NH * H))
        in_maps.append({
            "hidP": hidP,
            "wqP": ptile(wq_c),
            "wkP": ptile(wk_c),
            "wvP": ptile(wv_c),
            "woP": woP,
            "cosT": cosT,
            "sinA": sinA,
            "trimask": tri,
            "ident": ident,
        })
    return in_maps


def _run(in_maps, **kwargs):
    return run_bass_kernel_spmd(_get_nc(), in_maps,
                                core_ids=list(range(NCORES)), **kwargs)


def _gather(res):
    out = np.zeros((S, H), dtype=np.float32)
    for c in range(NCORES):
        out += np.asarray(res.results[c]["out"], dtype=np.float32)
    return out[None]


def kernel(hidden_states, attention_mask=None, wq=None, wk=None, wv=None, wo=None):
    in_maps = _prepare_in_maps(hidden_states, wq, wk, wv, wo)
    return _gather(_run(in_maps))
